# revision 36
# baseline (speedup 1.0000x reference)
"""ACM-GCN (2-layer) distributed Bass kernel for one TRN2 chip (8 NeuronCores).

Self-contained: takes FULL inputs (as produced by the problem's setup_inputs),
returns the FULL [N, C] float32 output.

Algorithm notes
---------------
Reference per layer (h: [N,F] node features, graph row=dest col=src):
    a        = inv_deg * segment_sum(h[col], row)        # row-normalized aggregate
    out_low  = relu(a @ Wl)
    out_high = relu((h - a) @ Wh)
    out_mlp  = relu(h @ Wm)
    logits   = [out_low@vl | out_high@vh | out_mlp@vm]   # [N,3]
    att      = softmax(sigmoid(logits) @ av / 3)
    out      = 3*(att0*out_low + att1*out_high + att2*out_mlp)
(uses segsum((h@W)[col]) == segsum(h[col]) @ W to do ONE gather per layer)

Distribution: nodes row-sharded over 8 cores (padded to NLPAD each); edges
partitioned by destination; full (padded) x replicated per core for the
layer-1 gather; AllGather of bf16 fea between layers.

Device mapping: per destination block-pair (256 rows) the gathered edge tiles
[128 edges x F] are contracted on TensorE against DVE-built selection
matrices onehot(seg)[128 x 256] (edge -> local dest, inv_deg pre-folded into
the gathered rows), accumulating a^T [F x 256] in PSUM.  All dense work is
done in transposed orientation [feat x nodes] so no activation transposes are
needed; bf16 matmul operands, fp32 PSUM accumulation.
"""

import math
from dataclasses import dataclass, field

import numpy as np
import ml_dtypes

BF16 = ml_dtypes.bfloat16


# ---------------------------------------------------------------- config ----


@dataclass
class Cfg:
    N: int = 100000
    E: int = 1600000
    F: int = 128
    H: int = 128
    C: int = 64
    NCORES: int = 8
    CHUNKS: int = 4
    DSPAN: int = 256          # dest rows per PSUM accumulation group (block pair)
    SB_PAIRS: int = 4         # pairs per gather superblock (keep == PG)
    PG: int = 4               # pairs per attention batch (ACT table amortization)
    CALL_TILES: int = 8       # max tiles (128 idx each) per dma_gather call:
                              # descs/engine = num_idxs/16+1 must stay < ring cap

    NL: int = field(init=False)
    NLPAD: int = field(init=False)
    NPAD: int = field(init=False)
    CHUNK_ROWS: int = field(init=False)
    PAIRS: int = field(init=False)

    def __post_init__(self):
        assert self.N % self.NCORES == 0
        self.NL = self.N // self.NCORES
        self.NLPAD = ((self.NL + self.DSPAN - 1) // self.DSPAN) * self.DSPAN
        self.NPAD = self.NLPAD * self.NCORES
        assert self.NPAD % self.CHUNKS == 0
        self.CHUNK_ROWS = self.NPAD // self.CHUNKS
        assert self.CHUNK_ROWS < 32768, "dma_gather int16 index range"
        self.PAIRS = self.NLPAD // self.DSPAN
        assert self.F == 128 and self.H == 128


@dataclass
class Sched:
    T: np.ndarray            # [PAIRS, CHUNKS] tiles per (pair, chunk), >=1
    tile_off: np.ndarray     # [PAIRS, CHUNKS] tile offset in global order
    TT: int                  # total tiles
    sbs: list                # list of lists of pair indices
    calls: list              # per (sb, c): (sb_idx, c, t0, t1)
    pgs: list                # list of lists of pair indices


def make_sched(cfg: Cfg, cnt_kpc: np.ndarray) -> Sched:
    """cnt_kpc: [NCORES, PAIRS, CHUNKS] edge counts."""
    T = np.maximum((cnt_kpc.max(axis=0) + 127) // 128, 1).astype(np.int64)
    sbs = [
        list(range(i, min(i + cfg.SB_PAIRS, cfg.PAIRS)))
        for i in range(0, cfg.PAIRS, cfg.SB_PAIRS)
    ]
    tile_off = np.zeros_like(T)
    calls = []
    off = 0
    for si, sb in enumerate(sbs):
        for c in range(cfg.CHUNKS):
            t0 = off
            for p in sb:
                tile_off[p, c] = off
                off += T[p, c]
            # split into dma_gather calls of <= CALL_TILES tiles each
            for s in range(t0, off, cfg.CALL_TILES):
                calls.append((si, c, s, min(s + cfg.CALL_TILES, off)))
    pgs = [
        list(range(i, min(i + cfg.PG, cfg.PAIRS)))
        for i in range(0, cfg.PAIRS, cfg.PG)
    ]
    return Sched(T=T, tile_off=tile_off, TT=int(off), sbs=sbs, calls=calls, pgs=pgs)


SEG_PAD = 384.0  # exactly representable in bf16, never matches iota 0..DSPAN-1


# ---------------------------------------------------------- preprocessing ----


def preprocess(cfg: Cfg, inputs: dict):
    """Returns (in_maps, sched). in_maps: per-core dict of named np arrays."""
    x = np.asarray(inputs["x"], np.float32)
    ei = np.asarray(inputs["edge_index"], np.int64)
    row, col = ei[0], ei[1]
    E = row.shape[0]

    owner = row // cfg.NL
    r_local = row % cfg.NL
    pair = r_local // cfg.DSPAN
    seg = (r_local % cfg.DSPAN).astype(np.float32)
    colp = (col // cfg.NL) * cfg.NLPAD + (col % cfg.NL)
    chunk = colp // cfg.CHUNK_ROWS
    col_loc = (colp % cfg.CHUNK_ROWS).astype(np.int16)

    deg = np.bincount(row, minlength=cfg.N).astype(np.float32)
    inv_deg = np.where(deg > 0, 1.0 / np.maximum(deg, 1.0), 0.0).astype(np.float32)
    val = inv_deg[row]

    key = (owner * cfg.PAIRS + pair) * cfg.CHUNKS + chunk
    nkeys = cfg.NCORES * cfg.PAIRS * cfg.CHUNKS
    cnt = np.bincount(key, minlength=nkeys)
    sched = make_sched(cfg, cnt.reshape(cfg.NCORES, cfg.PAIRS, cfg.CHUNKS))

    # rank of each edge within its (core, pair, chunk) group
    sk = np.argsort(key, kind="stable")
    starts = np.concatenate([[0], np.cumsum(cnt)[:-1]])
    rank = np.empty(E, np.int64)
    rank[sk] = np.arange(E) - starts[key[sk]]

    TT = sched.TT
    slot = owner * (TT * 128) + sched.tile_off.reshape(-1)[
        pair * cfg.CHUNKS + chunk
    ] * 128 + rank

    idx_all = np.zeros(cfg.NCORES * TT * 128, np.int16)
    seg_all = np.full(cfg.NCORES * TT * 128, SEG_PAD, np.float32)
    idx_all[slot] = col_loc
    seg_all[slot] = seg

    idx_all = idx_all.reshape(cfg.NCORES, TT, 128)
    seg_all = seg_all.reshape(cfg.NCORES, TT, 128)

    # gather-call-wrapped idx layout: per call tiles [t0,t1): flat -> [16, n/16],
    # replicated to 128 partitions; call occupies cols [t0*8, t1*8)
    idx_dram = np.zeros((cfg.NCORES, 128, TT * 8), np.int16)
    for (_, _, t0, t1) in sched.calls:
        for k in range(cfg.NCORES):
            flat = idx_all[k, t0:t1].reshape(-1)
            wrapped = flat.reshape(-1, 16).T  # [16, n/16]
            idx_dram[k, :, t0 * 8:t1 * 8] = np.tile(wrapped, (8, 1))

    seg_dram = np.ascontiguousarray(
        seg_all.transpose(0, 2, 1)).astype(BF16)  # [K,128,TT]

    # x, padded and bf16
    x_pad = np.zeros((cfg.NPAD, cfg.F), np.float32)
    for k in range(cfg.NCORES):
        x_pad[k * cfg.NLPAD:k * cfg.NLPAD + cfg.NL] = x[k * cfg.NL:(k + 1) * cfg.NL]
    x_gather = x_pad.astype(BF16)

    def w(name):
        return np.asarray(inputs[name], np.float32)

    iota = np.tile(np.arange(cfg.DSPAN, dtype=np.float32), (128, 1)).astype(BF16)
    onesr = np.ones((1, 128), np.float32).astype(BF16)
    third3 = np.full((3, 1), 1.0 / 3.0, np.float32).astype(BF16)
    sel3 = np.zeros((3, 384), np.float32)
    for i in range(3):
        sel3[i, i * 128:(i + 1) * 128] = 1.0
    sel3 = sel3.astype(BF16)
    id_bf = np.eye(128, dtype=np.float32).astype(BF16)
    id_f32 = np.eye(128, dtype=np.float32)

    invd_own = np.zeros((cfg.NCORES, 1, cfg.NLPAD), np.float32)
    for k in range(cfg.NCORES):
        invd_own[k, 0, :cfg.NL] = inv_deg[k * cfg.NL:(k + 1) * cfg.NL]

    in_maps = []
    for k in range(cfg.NCORES):
        xT_own = np.zeros((128, cfg.NLPAD), np.float32)
        xT_own[:, :cfg.NL] = x[k * cfg.NL:(k + 1) * cfg.NL].T
        m = {
            "x_gather": x_gather,
            "xT_own": xT_own.astype(BF16),
            "idx": idx_dram[k],
            "seg": seg_dram[k],
            "invd": invd_own[k],
            "Wl1": w("W_low1").astype(BF16), "Wh1": w("W_high1").astype(BF16),
            "Wm1": w("W_mlp1").astype(BF16),
            "av1": w("att1").astype(BF16),
            "Wl2": w("W_low2").astype(BF16), "Wh2": w("W_high2").astype(BF16),
            "Wm2": w("W_mlp2").astype(BF16),
            "av2": w("att2").astype(BF16),
            "iota": iota, "onesr": onesr, "third3": third3, "sel3": sel3,
            "id_bf": id_bf, "id_f32": id_f32,
        }
        # block-padded attention vectors: VL=[vl|0|0], VH=[0|vh|0], VM=[0|0|vm]
        # (logits computed as 3 PSUM-accumulating matmuls into one [3,*] tile)
        for li, names in (("1", ("v_low1", "v_high1", "v_mlp1")),
                          ("2", ("v_low2", "v_high2", "v_mlp2"))):
            fo = w(names[0]).shape[0]
            for i, nm in enumerate(("VL", "VH", "VM")):
                M = np.zeros((fo, 3), np.float32)
                M[:, i] = w(names[i])[:, 0]
                m[nm + li] = M.astype(BF16)
        in_maps.append(m)
    return in_maps, sched


# ----------------------------------------------------------------- builder ----


def build(cfg: Cfg, sched: Sched):
    from concourse import bacc, tile, mybir
    from contextlib import ExitStack

    dt = mybir.dt
    # 4 SWDGE queues: gathers round-robin across independent descriptor rings
    # so HBM random-read latency pipelines ~4 deep across the SDMA engines.
    nc = bacc.Bacc("TRN2", num_swdge_queues=4)
    qctr = [0]
    TT = sched.TT
    D = cfg.DSPAN
    F, H, C = cfg.F, cfg.H, cfg.C

    xg = nc.dram_tensor("x_gather", [cfg.NPAD, F], dt.bfloat16, kind="ExternalInput")
    xT = nc.dram_tensor("xT_own", [128, cfg.NLPAD], dt.bfloat16, kind="ExternalInput")
    idxT = nc.dram_tensor("idx", [128, TT * 8], dt.int16, kind="ExternalInput")
    segT = nc.dram_tensor("seg", [128, TT], dt.bfloat16, kind="ExternalInput")
    invdT = nc.dram_tensor("invd", [1, cfg.NLPAD], dt.float32, kind="ExternalInput")
    Wd = {}
    for li, (fi, fo) in (("1", (F, H)), ("2", (H, C))):
        for nme in ("Wl", "Wh", "Wm"):
            Wd[nme + li] = nc.dram_tensor(
                nme + li, [fi, fo], dt.bfloat16, kind="ExternalInput")
        for nme in ("VL", "VH", "VM"):
            Wd[nme + li] = nc.dram_tensor(
                nme + li, [fo, 3], dt.bfloat16, kind="ExternalInput")
        Wd["av" + li] = nc.dram_tensor(
            "av" + li, [3, 3], dt.bfloat16, kind="ExternalInput")
    iotaT = nc.dram_tensor("iota", [128, D], dt.bfloat16, kind="ExternalInput")
    onesT = nc.dram_tensor("onesr", [1, 128], dt.bfloat16, kind="ExternalInput")
    thirdT = nc.dram_tensor("third3", [3, 1], dt.bfloat16, kind="ExternalInput")
    selT = nc.dram_tensor("sel3", [3, 384], dt.bfloat16, kind="ExternalInput")
    idbT = nc.dram_tensor("id_bf", [128, 128], dt.bfloat16, kind="ExternalInput")
    idfT = nc.dram_tensor("id_f32", [128, 128], dt.float32, kind="ExternalInput")
    outT = nc.dram_tensor("out", [cfg.NLPAD, C], dt.float32, kind="ExternalOutput")

    with tile.TileContext(nc) as tc, ExitStack() as ctx:
        const = ctx.enter_context(tc.tile_pool(name="const", bufs=1))
        perst = ctx.enter_context(tc.tile_pool(name="perst", bufs=1))
        # G / idx / seg tiles: 4 chunk-calls live per superblock + prefetch
        gpool = ctx.enter_context(tc.tile_pool(name="gpool", bufs=8))
        ipool = ctx.enter_context(tc.tile_pool(name="ipool", bufs=9))
        svpool = ctx.enter_context(tc.tile_pool(name="svpool", bufs=10))
        smpool = ctx.enter_context(tc.tile_pool(name="smpool", bufs=4))
        abpool = ctx.enter_context(tc.tile_pool(name="abpool", bufs=3))
        opool = ctx.enter_context(tc.tile_pool(name="opool", bufs=cfg.PG + 3))
        attp = ctx.enter_context(tc.tile_pool(name="attp", bufs=2))
        cpool = ctx.enter_context(tc.tile_pool(name="cpool", bufs=3))
        stpool = ctx.enter_context(tc.tile_pool(name="stpool", bufs=2))
        dram = ctx.enter_context(tc.tile_pool(name="dram", bufs=1, space="DRAM"))
        ps_agg = ctx.enter_context(tc.tile_pool(name="ps_agg", bufs=1, space="PSUM"))
        ps_big = ctx.enter_context(tc.tile_pool(name="ps_big", bufs=2, space="PSUM"))
        ps_sm = ctx.enter_context(tc.tile_pool(name="ps_sm", bufs=1, space="PSUM"))
        ps_tr = ctx.enter_context(tc.tile_pool(name="ps_tr", bufs=1, space="PSUM"))

        def load_const(tensor, shape, dtp=dt.bfloat16):
            t = const.tile(shape, dtp, tag=tensor.name)
            nc.sync.dma_start(t[:], tensor[:])
            return t

        iota_t = load_const(iotaT, [128, D])
        ones_t = load_const(onesT, [1, 128])
        third_t = load_const(thirdT, [3, 1])
        sel_t = load_const(selT, [3, 384])
        idb_t = load_const(idbT, [128, 128])
        idf_t = load_const(idfT, [128, 128], dt.float32)
        Wt = {k: load_const(v, list(v.shape)) for k, v in Wd.items()}

        hT1 = perst.tile([128, cfg.NLPAD], dt.bfloat16, tag="hT1")
        nc.sync.dma_start(hT1[:], xT[:])
        feaT = perst.tile([128, cfg.NLPAD], dt.bfloat16, tag="feaT")

        fea_rm = dram.tile([cfg.NLPAD, F], dt.bfloat16)
        fea_full = dram.tile([cfg.NPAD, F], dt.bfloat16)

        TMAXP = int(sched.T.max())
        GMAXT = int(max(
            sum(sched.T[p, c] for p in sb)
            for sb in sched.sbs for c in range(cfg.CHUNKS)))

        def layer(li, hT, gsrc, HOUT, emit_out):
            Wl, Wh, Wm = Wt[f"Wl{li}"], Wt[f"Wh{li}"], Wt[f"Wm{li}"]
            Vs = [Wt[f"VL{li}"], Wt[f"VH{li}"], Wt[f"VM{li}"]]
            av = Wt[f"av{li}"]
            sb_calls = {}  # sb_idx -> {c: (gtile, t0)}
            pend = {}      # pair -> (ol, oh, om)
            pg_att = {}    # pg_idx -> attw tile

            def emit_gathers(si):
                sb_calls[si] = {}
                sb = sched.sbs[si]
                src = gsrc
                its = {}
                for c in range(cfg.CHUNKS):
                    t0 = int(sched.tile_off[sb[0], c])
                    t1 = int(sched.tile_off[sb[-1], c] + sched.T[sb[-1], c])
                    it = ipool.tile([128, GMAXT * 8], dt.int16, tag="idx")
                    nc.sync.dma_start(it[:, :(t1 - t0) * 8],
                                      idxT[:, t0 * 8:t1 * 8])
                    its[c] = it
                for c in range(cfg.CHUNKS):
                    t0 = int(sched.tile_off[sb[0], c])
                    t1 = int(sched.tile_off[sb[-1], c] + sched.T[sb[-1], c])
                    nt = t1 - t0
                    g = gpool.tile([128, GMAXT, 128], dt.bfloat16, tag="g")
                    csrc = src[c * cfg.CHUNK_ROWS:(c + 1) * cfg.CHUNK_ROWS, :]
                    it = its[c]
                    for (sj, cc, s0, s1) in sched.calls:
                        if sj != si or cc != c:
                            continue
                        ns = s1 - s0
                        num = ns * 128
                        nc.gpsimd.dma_gather(
                            g[:, s0 - t0:s1 - t0, :], csrc,
                            it[:, (s0 - t0) * 8:(s1 - t0) * 8], num, num, F,
                            queue_num=qctr[0] % 4,
                            single_packet=False,
                        )
                        qctr[0] += 1
                    st = svpool.tile([128, GMAXT], dt.bfloat16, tag="seg")
                    nc.sync.dma_start(st[:, :nt], segT[:, t0:t1])
                    sb_calls[si][c] = (g, st, t0)

            def front(p, si):
                agg = ps_agg.tile([128, D], dt.float32, space="PSUM", tag="agg")
                total = int(sched.T[p].sum())
                cnt = 0
                for c in range(cfg.CHUNKS):
                    g, st, t0 = sb_calls[si][c]
                    nt = int(sched.T[p, c])
                    loc = int(sched.tile_off[p, c] - t0)
                    sm = smpool.tile([128, TMAXP * D], dt.bfloat16, tag="sm")
                    nc.vector.tensor_tensor(
                        out=sm[:, :nt * D].rearrange("p (t m) -> p t m", m=D),
                        in0=st[:, loc:loc + nt].to_broadcast([128, nt, D]),
                        in1=iota_t.rearrange("p (o m) -> p o m", o=1)
                        .to_broadcast([128, nt, D]),
                        op=mybir.AluOpType.is_equal,
                    )
                    for t in range(nt):
                        nc.tensor.matmul(
                            out=agg[:],
                            lhsT=g[:, loc + t, :],
                            rhs=sm[:, t * D:(t + 1) * D],
                            start=(cnt == 0), stop=(cnt == total - 1),
                        )
                        cnt += 1
                p0 = p * D
                ib = abpool.tile([128, D], dt.float32, tag="ib")
                nc.sync.dma_start(
                    ib[:], invdT[0:1, p0:p0 + D].to_broadcast([128, D]))
                a_bf = abpool.tile([128, D], dt.bfloat16, tag="a")
                nc.vector.tensor_tensor(out=a_bf[:], in0=agg[:], in1=ib[:],
                                        op=mybir.AluOpType.mult)
                b_bf = abpool.tile([128, D], dt.bfloat16, tag="b")
                nc.vector.tensor_tensor(
                    out=b_bf[:], in0=hT[:, p0:p0 + D], in1=a_bf[:],
                    op=mybir.AluOpType.subtract)
                dps = ps_big.tile([128, 3 * D], dt.float32, space="PSUM", tag="big")
                nc.tensor.matmul(out=dps[:HOUT, 0:D], lhsT=Wl[:], rhs=a_bf[:],
                                 start=True, stop=True)
                nc.tensor.matmul(out=dps[:HOUT, D:2 * D], lhsT=Wh[:], rhs=b_bf[:],
                                 start=True, stop=True)
                nc.tensor.matmul(out=dps[:HOUT, 2 * D:3 * D], lhsT=Wm[:],
                                 rhs=hT[:, p0:p0 + D], start=True, stop=True)
                os = []
                for i in range(3):
                    o = opool.tile([HOUT, D], dt.bfloat16, tag=f"o{i}")
                    nc.scalar.activation(
                        o[:], dps[:HOUT, i * D:(i + 1) * D],
                        mybir.ActivationFunctionType.Relu)
                    os.append(o)
                pend[p] = os
                return os

            def attention(pgi, pg):
                w = len(pg) * D
                lg = ps_sm.tile([3, cfg.PG * D], dt.float32, space="PSUM", tag="sm")
                for j, p in enumerate(pg):
                    for i in range(3):
                        nc.tensor.matmul(
                            out=lg[:, j * D:(j + 1) * D],
                            lhsT=Vs[i][:HOUT, :], rhs=pend[p][i][:],
                            start=(i == 0), stop=(i == 2))
                sig = attp.tile([3, cfg.PG * D], dt.bfloat16, tag="sig")
                nc.scalar.activation(
                    sig[:, :w], lg[:, :w], mybir.ActivationFunctionType.Sigmoid)
                aps = ps_sm.tile([3, cfg.PG * D], dt.float32, space="PSUM", tag="sm")
                for s0 in range(0, w, 512):
                    s1 = min(s0 + 512, w)
                    nc.tensor.matmul(out=aps[:, s0:s1], lhsT=av[:],
                                     rhs=sig[:, s0:s1], start=True, stop=True)
                ex = attp.tile([3, cfg.PG * D], dt.bfloat16, tag="ex")
                nc.scalar.activation(
                    ex[:, :w], aps[:, :w], mybir.ActivationFunctionType.Exp,
                    scale=1.0 / 3.0)
                se = ps_sm.tile([3, cfg.PG * D], dt.float32, space="PSUM", tag="sm")
                for s0 in range(0, w, 512):
                    s1 = min(s0 + 512, w)
                    nc.tensor.matmul(out=se[0:1, s0:s1], lhsT=third_t[:],
                                     rhs=ex[:, s0:s1], start=True, stop=True)
                rc = attp.tile([1, cfg.PG * D], dt.bfloat16, tag="rc")
                with nc.allow_low_precision(reason="softmax recip in bf16 is fine"):
                    nc.vector.reciprocal(out=rc[:, :w], in_=se[0:1, :w])
                rb = ps_sm.tile([3, cfg.PG * D], dt.float32, space="PSUM", tag="sm")
                for s0 in range(0, w, 512):
                    s1 = min(s0 + 512, w)
                    nc.tensor.matmul(out=rb[:, s0:s1], lhsT=ones_t[0:1, :3],
                                     rhs=rc[0:1, s0:s1], start=True, stop=True)
                rbs = attp.tile([3, cfg.PG * D], dt.bfloat16, tag="rbs")
                nc.vector.tensor_copy(out=rbs[:, :w], in_=rb[:, :w])
                attw = attp.tile([3, cfg.PG * D], dt.bfloat16, tag="attw")
                nc.vector.tensor_tensor(out=attw[:, :w], in0=ex[:, :w],
                                        in1=rbs[:, :w], op=mybir.AluOpType.mult)
                pg_att[pgi] = attw

            def combine(p, pgi, pgj):
                attw = pg_att[pgi]
                ol, oh, om = pend.pop(p)
                eps = ps_big.tile([128, 3 * D], dt.float32, space="PSUM", tag="big")
                for i in range(3):
                    nc.tensor.matmul(
                        out=eps[:HOUT, i * D:(i + 1) * D],
                        lhsT=sel_t[:, i * 128:i * 128 + HOUT],
                        rhs=attw[:, pgj * D:(pgj + 1) * D],
                        start=True, stop=True)
                eb = cpool.tile([128, 3 * D], dt.bfloat16, tag="eb")
                nc.vector.tensor_copy(out=eb[:HOUT, :], in_=eps[:HOUT, :])
                acc = cpool.tile([HOUT, D], dt.bfloat16, tag="acc")
                tmp = cpool.tile([HOUT, D], dt.bfloat16, tag="tmp")
                nc.vector.tensor_tensor(out=acc[:], in0=ol[:], in1=eb[:HOUT, 0:D],
                                        op=mybir.AluOpType.mult)
                nc.vector.tensor_tensor(out=tmp[:], in0=oh[:], in1=eb[:HOUT, D:2 * D],
                                        op=mybir.AluOpType.mult)
                nc.vector.tensor_tensor(out=acc[:], in0=acc[:], in1=tmp[:],
                                        op=mybir.AluOpType.add)
                nc.vector.tensor_tensor(out=tmp[:], in0=om[:],
                                        in1=eb[:HOUT, 2 * D:3 * D],
                                        op=mybir.AluOpType.mult)
                p0 = p * D
                if emit_out is None:
                    # layer 1: fea = relu(acc+tmp) -> feaT resident + row-major DRAM
                    nc.vector.tensor_tensor(out=acc[:], in0=acc[:], in1=tmp[:],
                                            op=mybir.AluOpType.add)
                    nc.scalar.activation(
                        feaT[:, p0:p0 + D], acc[:],
                        mybir.ActivationFunctionType.Relu)
                    trp = ps_tr.tile([128, D], dt.bfloat16, space="PSUM", tag="tr")
                    for b in range(2):
                        nc.tensor.transpose(
                            out=trp[:, b * 128:(b + 1) * 128],
                            in_=feaT[:, p0 + b * 128:p0 + (b + 1) * 128],
                            identity=idb_t[:])
                    stg = stpool.tile([128, D], dt.bfloat16, tag="stg")
                    nc.vector.tensor_copy(out=stg[:], in_=trp[:])
                    nc.sync.dma_start(
                        out=fea_rm[p0:p0 + D, :].rearrange("(b r) f -> r b f", b=2),
                        in_=stg[:].rearrange("p (b f) -> p b f", b=2))
                else:
                    acc2 = cpool.tile([HOUT, D], dt.float32, tag="acc2")
                    nc.vector.tensor_tensor(out=acc2[:], in0=acc[:], in1=tmp[:],
                                            op=mybir.AluOpType.add)
                    trp = ps_tr.tile([128, D], dt.float32, space="PSUM", tag="tr")
                    for b in range(2):
                        nc.tensor.transpose(
                            out=trp[:, b * HOUT:(b + 1) * HOUT],
                            in_=acc2[:, b * 128:(b + 1) * 128],
                            identity=idf_t[:HOUT, :HOUT])
                    stg = stpool.tile([128, 2 * HOUT], dt.float32, tag="stgo")
                    nc.vector.tensor_copy(out=stg[:], in_=trp[:, :2 * HOUT])
                    nc.sync.dma_start(
                        out=emit_out[p0:p0 + D, :].rearrange(
                            "(b r) c -> r b c", b=2),
                        in_=stg[:].rearrange("p (b c) -> p b c", b=2))

            pair_to_sb = {}
            for si, sb in enumerate(sched.sbs):
                for p in sb:
                    pair_to_sb[p] = si
            done_sb = set()
            for pgi, pg in enumerate(sched.pgs):
                for p in pg:
                    si = pair_to_sb[p]
                    if si not in done_sb:
                        emit_gathers(si)
                        done_sb.add(si)
                    front(p, si)
                attention(pgi, pg)
                for pgj, p in enumerate(pg):
                    combine(p, pgi, pgj)

        layer("1", hT1, xg, H, None)
        nc.gpsimd.collective_compute(
            "AllGather",
            mybir.AluOpType.bypass,
            replica_groups=[list(range(cfg.NCORES))],
            ins=[fea_rm.opt()],
            outs=[fea_full.opt()],
        )
        layer("2", feaT, fea_full, C, outT)

    nc.compile()
    return nc


# ------------------------------------------------------------------ runtime ----


def kernel(**inputs) -> np.ndarray:
    cfg = Cfg()
    in_maps, sched = preprocess(cfg, inputs)
    nc = build(cfg, sched)
    from concourse.bass_utils import run_bass_kernel_spmd
    res = run_bass_kernel_spmd(
        nc, in_maps, core_ids=list(range(cfg.NCORES)))
    out = np.concatenate(
        [res.results[k]["out"][:cfg.NL] for k in range(cfg.NCORES)], axis=0)
    return out.astype(np.float32)


# revision 37
# speedup vs baseline: 1.0589x; 1.0589x over previous
"""ACM-GCN (2-layer) distributed Bass kernel for one TRN2 chip (8 NeuronCores).

Self-contained: takes FULL inputs (as produced by the problem's setup_inputs),
returns the FULL [N, C] float32 output.

Algorithm notes
---------------
Reference per layer (h: [N,F] node features, graph row=dest col=src):
    a        = inv_deg * segment_sum(h[col], row)        # row-normalized aggregate
    out_low  = relu(a @ Wl)
    out_high = relu((h - a) @ Wh)
    out_mlp  = relu(h @ Wm)
    logits   = [out_low@vl | out_high@vh | out_mlp@vm]   # [N,3]
    att      = softmax(sigmoid(logits) @ av / 3)
    out      = 3*(att0*out_low + att1*out_high + att2*out_mlp)
(uses segsum((h@W)[col]) == segsum(h[col]) @ W to do ONE gather per layer)

Distribution: nodes row-sharded over 8 cores (padded to NLPAD each); edges
partitioned by destination; full (padded) x replicated per core for the
layer-1 gather; AllGather of bf16 fea between layers.

Device mapping: per destination block-pair (256 rows) the gathered edge tiles
[128 edges x F] are contracted on TensorE against DVE-built selection
matrices onehot(seg)[128 x 256] (edge -> local dest, inv_deg pre-folded into
the gathered rows), accumulating a^T [F x 256] in PSUM.  All dense work is
done in transposed orientation [feat x nodes] so no activation transposes are
needed; bf16 matmul operands, fp32 PSUM accumulation.
"""

import math
from dataclasses import dataclass, field

import numpy as np
import ml_dtypes

BF16 = ml_dtypes.bfloat16


# ---------------------------------------------------------------- config ----


@dataclass
class Cfg:
    N: int = 100000
    E: int = 1600000
    F: int = 128
    H: int = 128
    C: int = 64
    NCORES: int = 8
    CHUNKS: int = 4
    DSPAN: int = 256          # dest rows per PSUM accumulation group (block pair)
    SB_PAIRS: int = 4         # pairs per gather superblock (keep == PG)
    PG: int = 4               # pairs per attention batch (ACT table amortization)
    CALL_TILES: int = 8       # max tiles (128 idx each) per dma_gather call:
                              # descs/engine = num_idxs/16+1 must stay < ring cap

    NL: int = field(init=False)
    NLPAD: int = field(init=False)
    NPAD: int = field(init=False)
    CHUNK_ROWS: int = field(init=False)
    PAIRS: int = field(init=False)

    def __post_init__(self):
        assert self.N % self.NCORES == 0
        self.NL = self.N // self.NCORES
        self.NLPAD = ((self.NL + self.DSPAN - 1) // self.DSPAN) * self.DSPAN
        self.NPAD = self.NLPAD * self.NCORES
        assert self.NPAD % self.CHUNKS == 0
        self.CHUNK_ROWS = self.NPAD // self.CHUNKS
        assert self.CHUNK_ROWS < 32768, "dma_gather int16 index range"
        self.PAIRS = self.NLPAD // self.DSPAN
        assert self.F == 128 and self.H == 128


@dataclass
class Sched:
    T: np.ndarray            # [PAIRS, CHUNKS] tiles per (pair, chunk), >=1
    tile_off: np.ndarray     # [PAIRS, CHUNKS] tile offset in global order
    TT: int                  # total tiles
    sbs: list                # list of lists of pair indices
    calls: list              # per (sb, c): (sb_idx, c, t0, t1)
    pgs: list                # list of lists of pair indices


def make_sched(cfg: Cfg, cnt_kpc: np.ndarray) -> Sched:
    """cnt_kpc: [NCORES, PAIRS, CHUNKS] edge counts."""
    T = np.maximum((cnt_kpc.max(axis=0) + 127) // 128, 1).astype(np.int64)
    sbs = [
        list(range(i, min(i + cfg.SB_PAIRS, cfg.PAIRS)))
        for i in range(0, cfg.PAIRS, cfg.SB_PAIRS)
    ]
    tile_off = np.zeros_like(T)
    calls = []
    off = 0
    for si, sb in enumerate(sbs):
        for c in range(cfg.CHUNKS):
            t0 = off
            for p in sb:
                tile_off[p, c] = off
                off += T[p, c]
            # split into dma_gather calls of <= CALL_TILES tiles each
            for s in range(t0, off, cfg.CALL_TILES):
                calls.append((si, c, s, min(s + cfg.CALL_TILES, off)))
    pgs = [
        list(range(i, min(i + cfg.PG, cfg.PAIRS)))
        for i in range(0, cfg.PAIRS, cfg.PG)
    ]
    return Sched(T=T, tile_off=tile_off, TT=int(off), sbs=sbs, calls=calls, pgs=pgs)


SEG_PAD = 384.0  # exactly representable in bf16, never matches iota 0..DSPAN-1


# ---------------------------------------------------------- preprocessing ----


def preprocess(cfg: Cfg, inputs: dict):
    """Returns (in_maps, sched). in_maps: per-core dict of named np arrays."""
    x = np.asarray(inputs["x"], np.float32)
    ei = np.asarray(inputs["edge_index"], np.int64)
    row, col = ei[0], ei[1]
    E = row.shape[0]

    owner = row // cfg.NL
    r_local = row % cfg.NL
    pair = r_local // cfg.DSPAN
    seg = (r_local % cfg.DSPAN).astype(np.float32)
    colp = (col // cfg.NL) * cfg.NLPAD + (col % cfg.NL)
    chunk = colp // cfg.CHUNK_ROWS
    col_loc = (colp % cfg.CHUNK_ROWS).astype(np.int16)

    deg = np.bincount(row, minlength=cfg.N).astype(np.float32)
    inv_deg = np.where(deg > 0, 1.0 / np.maximum(deg, 1.0), 0.0).astype(np.float32)
    val = inv_deg[row]

    key = (owner * cfg.PAIRS + pair) * cfg.CHUNKS + chunk
    nkeys = cfg.NCORES * cfg.PAIRS * cfg.CHUNKS
    cnt = np.bincount(key, minlength=nkeys)
    sched = make_sched(cfg, cnt.reshape(cfg.NCORES, cfg.PAIRS, cfg.CHUNKS))

    # rank of each edge within its (core, pair, chunk) group
    sk = np.argsort(key, kind="stable")
    starts = np.concatenate([[0], np.cumsum(cnt)[:-1]])
    rank = np.empty(E, np.int64)
    rank[sk] = np.arange(E) - starts[key[sk]]

    TT = sched.TT
    slot = owner * (TT * 128) + sched.tile_off.reshape(-1)[
        pair * cfg.CHUNKS + chunk
    ] * 128 + rank

    idx_all = np.zeros(cfg.NCORES * TT * 128, np.int16)
    seg_all = np.full(cfg.NCORES * TT * 128, SEG_PAD, np.float32)
    idx_all[slot] = col_loc
    seg_all[slot] = seg

    idx_all = idx_all.reshape(cfg.NCORES, TT, 128)
    seg_all = seg_all.reshape(cfg.NCORES, TT, 128)

    # gather-call-wrapped idx layout: per call tiles [t0,t1): flat -> [16, n/16],
    # replicated to 128 partitions; call occupies cols [t0*8, t1*8)
    idx_dram = np.zeros((cfg.NCORES, 128, TT * 8), np.int16)
    for (_, _, t0, t1) in sched.calls:
        for k in range(cfg.NCORES):
            flat = idx_all[k, t0:t1].reshape(-1)
            wrapped = flat.reshape(-1, 16).T  # [16, n/16]
            idx_dram[k, :, t0 * 8:t1 * 8] = np.tile(wrapped, (8, 1))

    seg_dram = np.ascontiguousarray(
        seg_all.transpose(0, 2, 1)).astype(BF16)  # [K,128,TT]

    # x, padded and bf16
    x_pad = np.zeros((cfg.NPAD, cfg.F), np.float32)
    for k in range(cfg.NCORES):
        x_pad[k * cfg.NLPAD:k * cfg.NLPAD + cfg.NL] = x[k * cfg.NL:(k + 1) * cfg.NL]
    x_gather = x_pad.astype(BF16)

    def w(name):
        return np.asarray(inputs[name], np.float32)

    iota = np.tile(np.arange(cfg.DSPAN, dtype=np.float32), (128, 1)).astype(BF16)
    onesr = np.ones((1, 128), np.float32).astype(BF16)
    third3 = np.full((3, 1), 1.0 / 3.0, np.float32).astype(BF16)
    sel3 = np.zeros((3, 384), np.float32)
    for i in range(3):
        sel3[i, i * 128:(i + 1) * 128] = 1.0
    sel3 = sel3.astype(BF16)
    id_bf = np.eye(128, dtype=np.float32).astype(BF16)
    id_f32 = np.eye(128, dtype=np.float32)

    invd_own = np.zeros((cfg.NCORES, 1, cfg.NLPAD), np.float32)
    for k in range(cfg.NCORES):
        invd_own[k, 0, :cfg.NL] = inv_deg[k * cfg.NL:(k + 1) * cfg.NL]

    in_maps = []
    for k in range(cfg.NCORES):
        xT_own = np.zeros((128, cfg.NLPAD), np.float32)
        xT_own[:, :cfg.NL] = x[k * cfg.NL:(k + 1) * cfg.NL].T
        m = {
            "x_gather": x_gather,
            "xT_own": xT_own.astype(BF16),
            "idx": idx_dram[k],
            "seg": seg_dram[k],
            "invd": invd_own[k],
            "Wl1": w("W_low1").astype(BF16), "Wh1": w("W_high1").astype(BF16),
            "Wm1": w("W_mlp1").astype(BF16),
            "av1": w("att1").astype(BF16),
            "Wl2": w("W_low2").astype(BF16), "Wh2": w("W_high2").astype(BF16),
            "Wm2": w("W_mlp2").astype(BF16),
            "av2": w("att2").astype(BF16),
            "iota": iota, "onesr": onesr, "third3": third3, "sel3": sel3,
            "id_bf": id_bf, "id_f32": id_f32,
        }
        # block-padded attention vectors: VL=[vl|0|0], VH=[0|vh|0], VM=[0|0|vm]
        # (logits computed as 3 PSUM-accumulating matmuls into one [3,*] tile)
        for li, names in (("1", ("v_low1", "v_high1", "v_mlp1")),
                          ("2", ("v_low2", "v_high2", "v_mlp2"))):
            fo = w(names[0]).shape[0]
            for i, nm in enumerate(("VL", "VH", "VM")):
                M = np.zeros((fo, 3), np.float32)
                M[:, i] = w(names[i])[:, 0]
                m[nm + li] = M.astype(BF16)
        in_maps.append(m)
    return in_maps, sched


# ----------------------------------------------------------------- builder ----


def build(cfg: Cfg, sched: Sched):
    from concourse import bacc, tile, mybir
    from contextlib import ExitStack

    dt = mybir.dt
    # 4 SWDGE queues: gathers round-robin across independent descriptor rings
    # so HBM random-read latency pipelines ~4 deep across the SDMA engines.
    nc = bacc.Bacc("TRN2", num_swdge_queues=4)
    qctr = [0]
    TT = sched.TT
    D = cfg.DSPAN
    F, H, C = cfg.F, cfg.H, cfg.C

    xg = nc.dram_tensor("x_gather", [cfg.NPAD, F], dt.bfloat16, kind="ExternalInput")
    xT = nc.dram_tensor("xT_own", [128, cfg.NLPAD], dt.bfloat16, kind="ExternalInput")
    idxT = nc.dram_tensor("idx", [128, TT * 8], dt.int16, kind="ExternalInput")
    segT = nc.dram_tensor("seg", [128, TT], dt.bfloat16, kind="ExternalInput")
    invdT = nc.dram_tensor("invd", [1, cfg.NLPAD], dt.float32, kind="ExternalInput")
    Wd = {}
    for li, (fi, fo) in (("1", (F, H)), ("2", (H, C))):
        for nme in ("Wl", "Wh", "Wm"):
            Wd[nme + li] = nc.dram_tensor(
                nme + li, [fi, fo], dt.bfloat16, kind="ExternalInput")
        for nme in ("VL", "VH", "VM"):
            Wd[nme + li] = nc.dram_tensor(
                nme + li, [fo, 3], dt.bfloat16, kind="ExternalInput")
        Wd["av" + li] = nc.dram_tensor(
            "av" + li, [3, 3], dt.bfloat16, kind="ExternalInput")
    iotaT = nc.dram_tensor("iota", [128, D], dt.bfloat16, kind="ExternalInput")
    onesT = nc.dram_tensor("onesr", [1, 128], dt.bfloat16, kind="ExternalInput")
    thirdT = nc.dram_tensor("third3", [3, 1], dt.bfloat16, kind="ExternalInput")
    selT = nc.dram_tensor("sel3", [3, 384], dt.bfloat16, kind="ExternalInput")
    idbT = nc.dram_tensor("id_bf", [128, 128], dt.bfloat16, kind="ExternalInput")
    idfT = nc.dram_tensor("id_f32", [128, 128], dt.float32, kind="ExternalInput")
    outT = nc.dram_tensor("out", [cfg.NLPAD, C], dt.float32, kind="ExternalOutput")

    with tile.TileContext(nc) as tc, ExitStack() as ctx:
        const = ctx.enter_context(tc.tile_pool(name="const", bufs=1))
        perst = ctx.enter_context(tc.tile_pool(name="perst", bufs=1))
        # G / idx / seg tiles: 4 chunk-calls live per superblock + prefetch
        gpool = ctx.enter_context(tc.tile_pool(name="gpool", bufs=8))
        ipool = ctx.enter_context(tc.tile_pool(name="ipool", bufs=9))
        svpool = ctx.enter_context(tc.tile_pool(name="svpool", bufs=10))
        smpool = ctx.enter_context(tc.tile_pool(name="smpool", bufs=4))
        abpool = ctx.enter_context(tc.tile_pool(name="abpool", bufs=3))
        opool = ctx.enter_context(tc.tile_pool(name="opool", bufs=cfg.PG + 3))
        attp = ctx.enter_context(tc.tile_pool(name="attp", bufs=2))
        cpool = ctx.enter_context(tc.tile_pool(name="cpool", bufs=3))
        stpool = ctx.enter_context(tc.tile_pool(name="stpool", bufs=2))
        dram = ctx.enter_context(tc.tile_pool(name="dram", bufs=1, space="DRAM"))
        ps_agg = ctx.enter_context(tc.tile_pool(name="ps_agg", bufs=1, space="PSUM"))
        ps_big = ctx.enter_context(tc.tile_pool(name="ps_big", bufs=2, space="PSUM"))
        ps_sm = ctx.enter_context(tc.tile_pool(name="ps_sm", bufs=1, space="PSUM"))
        ps_tr = ctx.enter_context(tc.tile_pool(name="ps_tr", bufs=1, space="PSUM"))

        def load_const(tensor, shape, dtp=dt.bfloat16):
            t = const.tile(shape, dtp, tag=tensor.name)
            nc.sync.dma_start(t[:], tensor[:])
            return t

        iota_t = load_const(iotaT, [128, D])
        ones_t = load_const(onesT, [1, 128])
        third_t = load_const(thirdT, [3, 1])
        sel_t = load_const(selT, [3, 384])
        idb_t = load_const(idbT, [128, 128])
        idf_t = load_const(idfT, [128, 128], dt.float32)
        Wt = {k: load_const(v, list(v.shape)) for k, v in Wd.items()}

        hT1 = perst.tile([128, cfg.NLPAD], dt.bfloat16, tag="hT1")
        nc.sync.dma_start(hT1[:], xT[:])
        feaT = perst.tile([128, cfg.NLPAD], dt.bfloat16, tag="feaT")

        fea_rm = dram.tile([cfg.NLPAD, F], dt.bfloat16)
        fea_full = dram.tile([cfg.NPAD, F], dt.bfloat16)

        TMAXP = int(sched.T.max())
        GMAXT = int(max(
            sum(sched.T[p, c] for p in sb)
            for sb in sched.sbs for c in range(cfg.CHUNKS)))

        def layer(li, hT, gsrc, HOUT, emit_out):
            Wl, Wh, Wm = Wt[f"Wl{li}"], Wt[f"Wh{li}"], Wt[f"Wm{li}"]
            Vs = [Wt[f"VL{li}"], Wt[f"VH{li}"], Wt[f"VM{li}"]]
            av = Wt[f"av{li}"]
            sb_calls = {}  # sb_idx -> {c: (gtile, t0)}
            pend = {}      # pair -> (ol, oh, om)
            pg_att = {}    # pg_idx -> attw tile

            def emit_gathers(si):
                sb_calls[si] = {}
                sb = sched.sbs[si]
                src = gsrc
                its = {}
                for c in range(cfg.CHUNKS):
                    t0 = int(sched.tile_off[sb[0], c])
                    t1 = int(sched.tile_off[sb[-1], c] + sched.T[sb[-1], c])
                    it = ipool.tile([128, GMAXT * 8], dt.int16, tag="idx")
                    nc.sync.dma_start(it[:, :(t1 - t0) * 8],
                                      idxT[:, t0 * 8:t1 * 8])
                    its[c] = it
                for c in range(cfg.CHUNKS):
                    t0 = int(sched.tile_off[sb[0], c])
                    t1 = int(sched.tile_off[sb[-1], c] + sched.T[sb[-1], c])
                    nt = t1 - t0
                    g = gpool.tile([128, GMAXT, 128], dt.bfloat16, tag="g")
                    csrc = src[c * cfg.CHUNK_ROWS:(c + 1) * cfg.CHUNK_ROWS, :]
                    it = its[c]
                    for (sj, cc, s0, s1) in sched.calls:
                        if sj != si or cc != c:
                            continue
                        ns = s1 - s0
                        num = ns * 128
                        nc.gpsimd.dma_gather(
                            g[:, s0 - t0:s1 - t0, :], csrc,
                            it[:, (s0 - t0) * 8:(s1 - t0) * 8], num, num, F,
                            queue_num=qctr[0] % 4,
                        )
                        qctr[0] += 1
                    st = svpool.tile([128, GMAXT], dt.bfloat16, tag="seg")
                    nc.sync.dma_start(st[:, :nt], segT[:, t0:t1])
                    sb_calls[si][c] = (g, st, t0)

            def front(p, si):
                agg = ps_agg.tile([128, D], dt.float32, space="PSUM", tag="agg")
                total = int(sched.T[p].sum())
                cnt = 0
                for c in range(cfg.CHUNKS):
                    g, st, t0 = sb_calls[si][c]
                    nt = int(sched.T[p, c])
                    loc = int(sched.tile_off[p, c] - t0)
                    sm = smpool.tile([128, TMAXP * D], dt.bfloat16, tag="sm")
                    nc.vector.tensor_tensor(
                        out=sm[:, :nt * D].rearrange("p (t m) -> p t m", m=D),
                        in0=st[:, loc:loc + nt].to_broadcast([128, nt, D]),
                        in1=iota_t.rearrange("p (o m) -> p o m", o=1)
                        .to_broadcast([128, nt, D]),
                        op=mybir.AluOpType.is_equal,
                    )
                    for t in range(nt):
                        nc.tensor.matmul(
                            out=agg[:],
                            lhsT=g[:, loc + t, :],
                            rhs=sm[:, t * D:(t + 1) * D],
                            start=(cnt == 0), stop=(cnt == total - 1),
                        )
                        cnt += 1
                p0 = p * D
                ib = abpool.tile([128, D], dt.float32, tag="ib")
                nc.sync.dma_start(
                    ib[:], invdT[0:1, p0:p0 + D].to_broadcast([128, D]))
                a_bf = abpool.tile([128, D], dt.bfloat16, tag="a")
                nc.vector.tensor_tensor(out=a_bf[:], in0=agg[:], in1=ib[:],
                                        op=mybir.AluOpType.mult)
                b_bf = abpool.tile([128, D], dt.bfloat16, tag="b")
                nc.vector.tensor_tensor(
                    out=b_bf[:], in0=hT[:, p0:p0 + D], in1=a_bf[:],
                    op=mybir.AluOpType.subtract)
                dps = ps_big.tile([128, 3 * D], dt.float32, space="PSUM", tag="big")
                nc.tensor.matmul(out=dps[:HOUT, 0:D], lhsT=Wl[:], rhs=a_bf[:],
                                 start=True, stop=True)
                nc.tensor.matmul(out=dps[:HOUT, D:2 * D], lhsT=Wh[:], rhs=b_bf[:],
                                 start=True, stop=True)
                nc.tensor.matmul(out=dps[:HOUT, 2 * D:3 * D], lhsT=Wm[:],
                                 rhs=hT[:, p0:p0 + D], start=True, stop=True)
                os = []
                for i in range(3):
                    o = opool.tile([HOUT, D], dt.bfloat16, tag=f"o{i}")
                    nc.scalar.activation(
                        o[:], dps[:HOUT, i * D:(i + 1) * D],
                        mybir.ActivationFunctionType.Relu)
                    os.append(o)
                pend[p] = os
                return os

            def attention(pgi, pg):
                w = len(pg) * D
                lg = ps_sm.tile([3, cfg.PG * D], dt.float32, space="PSUM", tag="sm")
                for j, p in enumerate(pg):
                    for i in range(3):
                        nc.tensor.matmul(
                            out=lg[:, j * D:(j + 1) * D],
                            lhsT=Vs[i][:HOUT, :], rhs=pend[p][i][:],
                            start=(i == 0), stop=(i == 2))
                sig = attp.tile([3, cfg.PG * D], dt.bfloat16, tag="sig")
                nc.scalar.activation(
                    sig[:, :w], lg[:, :w], mybir.ActivationFunctionType.Sigmoid)
                aps = ps_sm.tile([3, cfg.PG * D], dt.float32, space="PSUM", tag="sm")
                for s0 in range(0, w, 512):
                    s1 = min(s0 + 512, w)
                    nc.tensor.matmul(out=aps[:, s0:s1], lhsT=av[:],
                                     rhs=sig[:, s0:s1], start=True, stop=True)
                ex = attp.tile([3, cfg.PG * D], dt.bfloat16, tag="ex")
                nc.scalar.activation(
                    ex[:, :w], aps[:, :w], mybir.ActivationFunctionType.Exp,
                    scale=1.0 / 3.0)
                se = ps_sm.tile([3, cfg.PG * D], dt.float32, space="PSUM", tag="sm")
                for s0 in range(0, w, 512):
                    s1 = min(s0 + 512, w)
                    nc.tensor.matmul(out=se[0:1, s0:s1], lhsT=third_t[:],
                                     rhs=ex[:, s0:s1], start=True, stop=True)
                rc = attp.tile([1, cfg.PG * D], dt.bfloat16, tag="rc")
                with nc.allow_low_precision(reason="softmax recip in bf16 is fine"):
                    nc.vector.reciprocal(out=rc[:, :w], in_=se[0:1, :w])
                rb = ps_sm.tile([3, cfg.PG * D], dt.float32, space="PSUM", tag="sm")
                for s0 in range(0, w, 512):
                    s1 = min(s0 + 512, w)
                    nc.tensor.matmul(out=rb[:, s0:s1], lhsT=ones_t[0:1, :3],
                                     rhs=rc[0:1, s0:s1], start=True, stop=True)
                rbs = attp.tile([3, cfg.PG * D], dt.bfloat16, tag="rbs")
                nc.vector.tensor_copy(out=rbs[:, :w], in_=rb[:, :w])
                attw = attp.tile([3, cfg.PG * D], dt.bfloat16, tag="attw")
                nc.vector.tensor_tensor(out=attw[:, :w], in0=ex[:, :w],
                                        in1=rbs[:, :w], op=mybir.AluOpType.mult)
                pg_att[pgi] = attw

            def combine(p, pgi, pgj):
                attw = pg_att[pgi]
                ol, oh, om = pend.pop(p)
                eps = ps_big.tile([128, 3 * D], dt.float32, space="PSUM", tag="big")
                for i in range(3):
                    nc.tensor.matmul(
                        out=eps[:HOUT, i * D:(i + 1) * D],
                        lhsT=sel_t[:, i * 128:i * 128 + HOUT],
                        rhs=attw[:, pgj * D:(pgj + 1) * D],
                        start=True, stop=True)
                eb = cpool.tile([128, 3 * D], dt.bfloat16, tag="eb")
                nc.vector.tensor_copy(out=eb[:HOUT, :], in_=eps[:HOUT, :])
                acc = cpool.tile([HOUT, D], dt.bfloat16, tag="acc")
                tmp = cpool.tile([HOUT, D], dt.bfloat16, tag="tmp")
                nc.vector.tensor_tensor(out=acc[:], in0=ol[:], in1=eb[:HOUT, 0:D],
                                        op=mybir.AluOpType.mult)
                nc.vector.tensor_tensor(out=tmp[:], in0=oh[:], in1=eb[:HOUT, D:2 * D],
                                        op=mybir.AluOpType.mult)
                nc.vector.tensor_tensor(out=acc[:], in0=acc[:], in1=tmp[:],
                                        op=mybir.AluOpType.add)
                nc.vector.tensor_tensor(out=tmp[:], in0=om[:],
                                        in1=eb[:HOUT, 2 * D:3 * D],
                                        op=mybir.AluOpType.mult)
                p0 = p * D
                if emit_out is None:
                    # layer 1: fea = relu(acc+tmp) -> feaT resident + row-major DRAM
                    nc.vector.tensor_tensor(out=acc[:], in0=acc[:], in1=tmp[:],
                                            op=mybir.AluOpType.add)
                    nc.scalar.activation(
                        feaT[:, p0:p0 + D], acc[:],
                        mybir.ActivationFunctionType.Relu)
                    trp = ps_tr.tile([128, D], dt.bfloat16, space="PSUM", tag="tr")
                    for b in range(2):
                        nc.tensor.transpose(
                            out=trp[:, b * 128:(b + 1) * 128],
                            in_=feaT[:, p0 + b * 128:p0 + (b + 1) * 128],
                            identity=idb_t[:])
                    stg = stpool.tile([128, D], dt.bfloat16, tag="stg")
                    nc.vector.tensor_copy(out=stg[:], in_=trp[:])
                    nc.sync.dma_start(
                        out=fea_rm[p0:p0 + D, :].rearrange("(b r) f -> r b f", b=2),
                        in_=stg[:].rearrange("p (b f) -> p b f", b=2))
                else:
                    acc2 = cpool.tile([HOUT, D], dt.float32, tag="acc2")
                    nc.vector.tensor_tensor(out=acc2[:], in0=acc[:], in1=tmp[:],
                                            op=mybir.AluOpType.add)
                    trp = ps_tr.tile([128, D], dt.float32, space="PSUM", tag="tr")
                    for b in range(2):
                        nc.tensor.transpose(
                            out=trp[:, b * HOUT:(b + 1) * HOUT],
                            in_=acc2[:, b * 128:(b + 1) * 128],
                            identity=idf_t[:HOUT, :HOUT])
                    stg = stpool.tile([128, 2 * HOUT], dt.float32, tag="stgo")
                    nc.vector.tensor_copy(out=stg[:], in_=trp[:, :2 * HOUT])
                    nc.sync.dma_start(
                        out=emit_out[p0:p0 + D, :].rearrange(
                            "(b r) c -> r b c", b=2),
                        in_=stg[:].rearrange("p (b c) -> p b c", b=2))

            pair_to_sb = {}
            for si, sb in enumerate(sched.sbs):
                for p in sb:
                    pair_to_sb[p] = si
            done_sb = set()
            for pgi, pg in enumerate(sched.pgs):
                for p in pg:
                    si = pair_to_sb[p]
                    if si not in done_sb:
                        emit_gathers(si)
                        done_sb.add(si)
                    front(p, si)
                attention(pgi, pg)
                for pgj, p in enumerate(pg):
                    combine(p, pgi, pgj)

        layer("1", hT1, xg, H, None)
        nc.gpsimd.collective_compute(
            "AllGather",
            mybir.AluOpType.bypass,
            replica_groups=[list(range(cfg.NCORES))],
            ins=[fea_rm.opt()],
            outs=[fea_full.opt()],
        )
        layer("2", feaT, fea_full, C, outT)

    nc.compile()
    return nc


# ------------------------------------------------------------------ runtime ----


def kernel(**inputs) -> np.ndarray:
    cfg = Cfg()
    in_maps, sched = preprocess(cfg, inputs)
    nc = build(cfg, sched)
    from concourse.bass_utils import run_bass_kernel_spmd
    res = run_bass_kernel_spmd(
        nc, in_maps, core_ids=list(range(cfg.NCORES)))
    out = np.concatenate(
        [res.results[k]["out"][:cfg.NL] for k in range(cfg.NCORES)], axis=0)
    return out.astype(np.float32)
